# revision 40
# baseline (speedup 1.0000x reference)
"""Trainium2 Bass kernel for: out[b,h,w,i,j] = inputs[b,h,w,i] * u[i,j]
where u = beta^2 / rowsum(beta^2), beta [32,2].

Full shapes: inputs [16,160,320,32] f32, beta [32,2] f32,
out [16,160,320,32,2] f32.

Sharding: pure data parallel over batch B=16 across 8 cores (2 batches
per core); beta replicated.

Per-core layout: the input shard (2*160*320*32 = 3,276,800 floats,
contiguous) is viewed as [128 partitions, 25600]. Since 25600 % 32 == 0
each partition covers whole rows of 32 channels, so the matching output
shard view is [128, 51200] with out_col = 2*in_col alignment.

Raw Bass (explicit semaphores): the Tile auto-sync layer emits >1
sync-wait per compute/DMA instruction, which this walrus build rejects
("Too many sync wait commands"), so waits are standalone wait_ge
instructions on each engine's stream.

Pipeline (per core): N_CHUNKS chunks (tapered sizes), NBUF-buffered.
  sync engine:   input DMAs on the qSP HWDGE ring
  scalar engine: output DMAs on the qAct HWDGE ring
  vector engine: [w beta] setup-u; for k: [w in_k] [w out_{k-NBUF}] mul_k
  gpsimd:        broadcast-load beta into all partitions
"""

import numpy as np

import concourse.bass as bass
import concourse.mybir as mybir
from concourse.bass_utils import run_bass_kernel_spmd

N_CORES = 8
P = 128
D = 32  # channels
C = 2  # beta columns
COLS_IN = 25600  # per-partition input elements (per core)
COLS_OUT = 2 * COLS_IN
# Input-column chunk sizes (each % 32 == 0, summing to COLS_IN).  The
# first chunks taper up so mul0 finishes early and the first out-DMA
# enqueues before the input stream would otherwise leave the DMA ring
# idle waiting on it.
CHUNK_COLS = [640, 1280, 1920, 2560, 3200, 3200, 3200, 3200, 3200, 3200]
assert sum(CHUNK_COLS) == COLS_IN and all(c % D == 0 for c in CHUNK_COLS)
CHUNK_OFF = [sum(CHUNK_COLS[:k]) for k in range(len(CHUNK_COLS))]
N_CHUNKS = len(CHUNK_COLS)
G_MAX = max(CHUNK_COLS) // D
NBUF = 4

FULL_IN_SHAPE = (16, 160, 320, 32)
FULL_OUT_SHAPE = (16, 160, 320, 32, 2)

_cached = {}


def _build_program():
    from contextlib import ExitStack

    f32 = mybir.dt.float32
    nc = bass.Bass()
    x = nc.declare_dram_parameter("inputs", [P, COLS_IN], f32, isOutput=False)
    beta = nc.declare_dram_parameter("beta", [D, C], f32, isOutput=False)
    y = nc.declare_dram_parameter("out", [P, COLS_OUT], f32, isOutput=True)

    stack = ExitStack()

    b = stack.enter_context(nc.sbuf_tensor("b_bcast", [P, D, C], f32))
    bsq = stack.enter_context(nc.sbuf_tensor("bsq", [P, D, C], f32))
    s = stack.enter_context(nc.sbuf_tensor("s", [P, D], f32))
    r = stack.enter_context(nc.sbuf_tensor("r", [P, D], f32))
    u = stack.enter_context(nc.sbuf_tensor("u", [P, D, C], f32))
    xin = [
        stack.enter_context(nc.sbuf_tensor(f"xin{i}", [P, G_MAX, D], f32))
        for i in range(NBUF)
    ]
    yout = [
        stack.enter_context(nc.sbuf_tensor(f"yout{i}", [P, G_MAX, D, C], f32))
        for i in range(NBUF)
    ]

    bsem = nc.alloc_semaphore("bsem")
    # One semaphore per buffer slot for each DMA stream.  A single shared
    # sem with "wait >= 16*(k+1)" is UNSOUND: each of the 16 SDMA engines
    # increments by 1 per DMA, so the total count can reach 16*(k+1) with
    # a lagging engine still moving chunk k while fast engines raced
    # ahead through later chunks.  Per-slot sems have at most one DMA in
    # flight each (slot reuse is gated on the consumer), so the
    # threshold is exact.
    in_sems = [nc.alloc_semaphore(f"in_sem{i}") for i in range(NBUF)]
    out_sems = [nc.alloc_semaphore(f"out_sem{i}") for i in range(NBUF)]
    mul_sem = nc.alloc_semaphore("mul_sem")

    def in_dma(k, eng=None):
        c0, nc_cols = CHUNK_OFF[k], CHUNK_COLS[k]
        (eng or nc.sync).dma_start(
            out=xin[k % NBUF][:, : nc_cols // D, :],
            in_=x[:, c0 : c0 + nc_cols].rearrange("p (g i) -> p g i", i=D),
        ).then_inc(in_sems[k % NBUF], 16)

    # gpsimd stream: broadcast beta [32,2] into every partition.
    beta_ap = beta[:, :]
    beta_bcast = bass.AP(
        tensor=beta_ap.tensor,
        offset=beta_ap.offset,
        ap=[[0, P], *beta_ap.ap],
    )
    nc.gpsimd.dma_start(out=b[:], in_=beta_bcast).then_inc(bsem, 16)

    def out_dma(k, eng=None):
        c0, nc_cols = CHUNK_OFF[k], CHUNK_COLS[k]
        (eng or nc.scalar).dma_start(
            out=y[:, 2 * c0 : 2 * (c0 + nc_cols)].rearrange(
                "p (g i j) -> p g i j", i=D, j=C
            ),
            in_=yout[k % NBUF][:, : nc_cols // D, :, :],
        ).then_inc(out_sems[k % NBUF], 16)

    # sync engine stream: all input DMAs (qSP HWDGE ring)
    for k in range(min(NBUF, N_CHUNKS)):
        in_dma(k)
    for k in range(N_CHUNKS - NBUF):
        nc.sync.wait_ge(mul_sem, k + 1)
        in_dma(k + NBUF)

    # scalar engine stream: output DMAs for slots 0-2 (qAct HWDGE ring);
    # slot-3 outs go via the gpsimd SWDGE queue as a third DMA queue.
    for k in range(N_CHUNKS):
        if k % NBUF == 3:
            continue
        nc.scalar.wait_ge(mul_sem, k + 1)
        out_dma(k)
    for slot in range(NBUF - 1):
        n_slot = len([k for k in range(N_CHUNKS) if k % NBUF == slot])
        nc.scalar.wait_ge(out_sems[slot], 16 * n_slot)

    # gpsimd stream (after the beta broadcast above): slot-3 outs
    for k in range(N_CHUNKS):
        if k % NBUF != 3:
            continue
        nc.gpsimd.wait_ge(mul_sem, k + 1)
        out_dma(k, eng=nc.gpsimd)
    n3 = len([k for k in range(N_CHUNKS) if k % NBUF == 3])
    nc.gpsimd.wait_ge(out_sems[3], 16 * n3)

    # vector engine stream.  DVE sem updates fire at instruction retire,
    # BEFORE the engine's SBUF write buffer drains — a cross-engine (or
    # even same-engine) consumer can observe stale trailing bytes.  So
    # every producer->consumer handoff of DVE output goes through an
    # explicit drain, with the signalling sem attached to the drain.
    v = nc.vector
    v.wait_ge(bsem, 16)
    v.tensor_mul(bsq[:], b[:], b[:])
    v.tensor_add(s[:], bsq[:, :, 0], bsq[:, :, 1])
    v.reciprocal(r[:], s[:])
    # reciprocal's writes are not visible to the next DVE op without an
    # explicit drain (same-engine RAW hazard).
    v.drain()
    v.tensor_tensor(
        u[:], r[:, :, None].to_broadcast([P, D, C]), bsq[:], mybir.AluOpType.mult
    )
    v.drain()
    for k in range(N_CHUNKS):
        g = CHUNK_COLS[k] // D
        v.wait_ge(in_sems[k % NBUF], 16 * (k // NBUF + 1))
        if k >= NBUF:
            v.wait_ge(out_sems[k % NBUF], 16 * (k // NBUF))
        v.tensor_tensor(
            yout[k % NBUF][:, :g, :, :],
            xin[k % NBUF][:, :g, :, None].to_broadcast([P, g, D, C]),
            u[:, None, :, :].to_broadcast([P, g, D, C]),
            mybir.AluOpType.mult,
        )
        v.drain().then_inc(mul_sem, 1)

    _cached["stack"] = stack
    return nc


def get_program():
    if "nc" not in _cached:
        _cached["nc"] = _build_program()
    return _cached["nc"]


def kernel(inputs: np.ndarray, beta: np.ndarray) -> np.ndarray:
    assert inputs.shape == FULL_IN_SHAPE, inputs.shape
    inputs = np.ascontiguousarray(inputs, dtype=np.float32)
    beta = np.ascontiguousarray(beta, dtype=np.float32)

    shards = inputs.reshape(N_CORES, P, COLS_IN)
    in_maps = [{"inputs": shards[c], "beta": beta} for c in range(N_CORES)]

    nc = get_program()
    res = run_bass_kernel_spmd(nc, in_maps, list(range(N_CORES))).results

    out = np.empty((N_CORES, P, COLS_OUT), dtype=np.float32)
    for c in range(N_CORES):
        out[c] = res[c]["out"]
    return out.reshape(FULL_OUT_SHAPE)


# revision 41
# speedup vs baseline: 1.0267x; 1.0267x over previous
"""Trainium2 Bass kernel for: out[b,h,w,i,j] = inputs[b,h,w,i] * u[i,j]
where u = beta^2 / rowsum(beta^2), beta [32,2].

Full shapes: inputs [16,160,320,32] f32, beta [32,2] f32,
out [16,160,320,32,2] f32.

Sharding: pure data parallel over batch B=16 across 8 cores (2 batches
per core); beta replicated.

Per-core layout: the input shard (2*160*320*32 = 3,276,800 floats,
contiguous) is viewed as [128 partitions, 25600]. Since 25600 % 32 == 0
each partition covers whole rows of 32 channels, so the matching output
shard view is [128, 51200] with out_col = 2*in_col alignment.

Raw Bass (explicit semaphores): the Tile auto-sync layer emits >1
sync-wait per compute/DMA instruction, which this walrus build rejects
("Too many sync wait commands"), so waits are standalone wait_ge
instructions on each engine's stream.

Pipeline (per core): N_CHUNKS chunks (tapered sizes), NBUF-buffered.
  sync engine:   input DMAs on the qSP HWDGE ring
  scalar engine: output DMAs on the qAct HWDGE ring
  vector engine: [w beta] setup-u; for k: [w in_k] [w out_{k-NBUF}] mul_k
  gpsimd:        broadcast-load beta into all partitions
"""

import numpy as np

import concourse.bass as bass
import concourse.mybir as mybir
from concourse.bass_utils import run_bass_kernel_spmd

N_CORES = 8
P = 128
D = 32  # channels
C = 2  # beta columns
COLS_IN = 25600  # per-partition input elements (per core)
COLS_OUT = 2 * COLS_IN
# Input-column chunk sizes (each % 32 == 0, summing to COLS_IN).  The
# first chunks taper up so mul0 finishes early and the first out-DMA
# enqueues before the input stream would otherwise leave the DMA ring
# idle waiting on it.
CHUNK_COLS = [640, 1280, 1920, 2560, 3200, 3200, 3200, 3200, 3200, 3200]
assert sum(CHUNK_COLS) == COLS_IN and all(c % D == 0 for c in CHUNK_COLS)
CHUNK_OFF = [sum(CHUNK_COLS[:k]) for k in range(len(CHUNK_COLS))]
N_CHUNKS = len(CHUNK_COLS)
G_MAX = max(CHUNK_COLS) // D
NBUF = 4

FULL_IN_SHAPE = (16, 160, 320, 32)
FULL_OUT_SHAPE = (16, 160, 320, 32, 2)

_cached = {}


def _build_program():
    from contextlib import ExitStack

    f32 = mybir.dt.float32
    nc = bass.Bass(dynamic_dma_scratch_size=8192)
    x = nc.declare_dram_parameter("inputs", [P, COLS_IN], f32, isOutput=False)
    beta = nc.declare_dram_parameter("beta", [D, C], f32, isOutput=False)
    y = nc.declare_dram_parameter("out", [P, COLS_OUT], f32, isOutput=True)

    stack = ExitStack()

    b = stack.enter_context(nc.sbuf_tensor("b_bcast", [P, D, C], f32))
    bsq = stack.enter_context(nc.sbuf_tensor("bsq", [P, D, C], f32))
    s = stack.enter_context(nc.sbuf_tensor("s", [P, D], f32))
    r = stack.enter_context(nc.sbuf_tensor("r", [P, D], f32))
    u = stack.enter_context(nc.sbuf_tensor("u", [P, D, C], f32))
    xin = [
        stack.enter_context(nc.sbuf_tensor(f"xin{i}", [P, G_MAX, D], f32))
        for i in range(NBUF)
    ]
    yout = [
        stack.enter_context(nc.sbuf_tensor(f"yout{i}", [P, G_MAX, D, C], f32))
        for i in range(NBUF)
    ]

    bsem = nc.alloc_semaphore("bsem")
    # One semaphore per buffer slot for each DMA stream.  A single shared
    # sem with "wait >= 16*(k+1)" is UNSOUND: each of the 16 SDMA engines
    # increments by 1 per DMA, so the total count can reach 16*(k+1) with
    # a lagging engine still moving chunk k while fast engines raced
    # ahead through later chunks.  Per-slot sems have at most one DMA in
    # flight each (slot reuse is gated on the consumer), so the
    # threshold is exact.
    in_sems = [nc.alloc_semaphore(f"in_sem{i}") for i in range(NBUF)]
    out_sems = [nc.alloc_semaphore(f"out_sem{i}") for i in range(NBUF)]
    mul_sem = nc.alloc_semaphore("mul_sem")

    def in_dma(k, eng=None):
        c0, nc_cols = CHUNK_OFF[k], CHUNK_COLS[k]
        (eng or nc.sync).dma_start(
            out=xin[k % NBUF][:, : nc_cols // D, :],
            in_=x[:, c0 : c0 + nc_cols].rearrange("p (g i) -> p g i", i=D),
        ).then_inc(in_sems[k % NBUF], 16)

    # gpsimd stream: broadcast beta [32,2] into every partition.
    beta_ap = beta[:, :]
    beta_bcast = bass.AP(
        tensor=beta_ap.tensor,
        offset=beta_ap.offset,
        ap=[[0, P], *beta_ap.ap],
    )
    nc.gpsimd.dma_start(out=b[:], in_=beta_bcast).then_inc(bsem, 16)

    def out_dma(k, eng=None):
        c0, nc_cols = CHUNK_OFF[k], CHUNK_COLS[k]
        (eng or nc.scalar).dma_start(
            out=y[:, 2 * c0 : 2 * (c0 + nc_cols)].rearrange(
                "p (g i j) -> p g i j", i=D, j=C
            ),
            in_=yout[k % NBUF][:, : nc_cols // D, :, :],
        ).then_inc(out_sems[k % NBUF], 16)

    # sync engine stream: all input DMAs (qSP HWDGE ring)
    for k in range(min(NBUF, N_CHUNKS)):
        in_dma(k)
    for k in range(N_CHUNKS - NBUF):
        nc.sync.wait_ge(mul_sem, k + 1)
        in_dma(k + NBUF)

    # scalar engine stream: all output DMAs (qAct HWDGE ring)
    for k in range(N_CHUNKS):
        nc.scalar.wait_ge(mul_sem, k + 1)
        out_dma(k)
    for slot in range(NBUF):
        n_slot = len([k for k in range(N_CHUNKS) if k % NBUF == slot])
        nc.scalar.wait_ge(out_sems[slot], 16 * n_slot)

    # vector engine stream.  DVE sem updates fire at instruction retire,
    # BEFORE the engine's SBUF write buffer drains — a cross-engine (or
    # even same-engine) consumer can observe stale trailing bytes.  So
    # every producer->consumer handoff of DVE output goes through an
    # explicit drain, with the signalling sem attached to the drain.
    v = nc.vector
    v.wait_ge(bsem, 16)
    v.tensor_mul(bsq[:], b[:], b[:])
    v.tensor_add(s[:], bsq[:, :, 0], bsq[:, :, 1])
    v.reciprocal(r[:], s[:])
    # reciprocal's writes are not visible to the next DVE op without an
    # explicit drain (same-engine RAW hazard).
    v.drain()
    v.tensor_tensor(
        u[:], r[:, :, None].to_broadcast([P, D, C]), bsq[:], mybir.AluOpType.mult
    )
    v.drain()
    for k in range(N_CHUNKS):
        g = CHUNK_COLS[k] // D
        v.wait_ge(in_sems[k % NBUF], 16 * (k // NBUF + 1))
        if k >= NBUF:
            v.wait_ge(out_sems[k % NBUF], 16 * (k // NBUF))
        v.tensor_tensor(
            yout[k % NBUF][:, :g, :, :],
            xin[k % NBUF][:, :g, :, None].to_broadcast([P, g, D, C]),
            u[:, None, :, :].to_broadcast([P, g, D, C]),
            mybir.AluOpType.mult,
        )
        v.drain().then_inc(mul_sem, 1)

    _cached["stack"] = stack
    return nc


def get_program():
    if "nc" not in _cached:
        _cached["nc"] = _build_program()
    return _cached["nc"]


def kernel(inputs: np.ndarray, beta: np.ndarray) -> np.ndarray:
    assert inputs.shape == FULL_IN_SHAPE, inputs.shape
    inputs = np.ascontiguousarray(inputs, dtype=np.float32)
    beta = np.ascontiguousarray(beta, dtype=np.float32)

    shards = inputs.reshape(N_CORES, P, COLS_IN)
    in_maps = [{"inputs": shards[c], "beta": beta} for c in range(N_CORES)]

    nc = get_program()
    res = run_bass_kernel_spmd(nc, in_maps, list(range(N_CORES))).results

    out = np.empty((N_CORES, P, COLS_OUT), dtype=np.float32)
    for c in range(N_CORES):
        out[c] = res[c]["out"]
    return out.reshape(FULL_OUT_SHAPE)
